# revision 22
# baseline (speedup 1.0000x reference)
"""Multi-head attention (B=1, N=4096, C=512, H=8) on 8 Trainium2 NeuronCores.

Tensor-parallel over heads: core h computes head h end-to-end (QKV proj,
softmax(q k^T) v, proj-slice), emitting the *unnormalized* projected partial
(softmax denominator deferred) plus per-query bf16 row sums; the host divides
and all-reduces (sums) the 8 partials and adds bproj.

Design (v3):
  - whole PE path is bf16 (weights, x, q/k/v, exp output).  PSUM accums stay
    fp32.  End-to-end rel err ~1e-2 vs the 2e-2 gate.
  - softmax exp (16.7M elements/core, the largest single engine load) is
    split between ScalarE (exact Exp activation) and the Vector engine,
    which computes a Schraudolph-style exp in ONE op:
    pt_bf16 = bitcast(int16(s*scale*128/ln2 + (127*128-5.5)))
    (the int16 bit pattern of 2^y IS the bf16 value; linear-frac approx has
    max multiplicative error ~3.3%, harmless under softmax normalization).
  - q^T/k^T are computed once (single [q|k] M=128 chunk per block) and
    duplicated into both partition halves by SBUF-SBUF DMA, so score matmuls
    run 2-way row-packed (K=64 pairs).  No separate q pass.
  - v tiles [128 keys, 64] come from DMA XBAR transposes into a contiguous
    staging tile (the XBAR writer needs contiguous output), then GpSimd
    scatters into the stride-65 [v | 1] tiles (ones column -> free row sums,
    M=65 av matmuls).
  - av lags scores/exp by 2 groups so both exp engines run concurrently.
  - PSUM: sc 2x2 banks + av 2x1 + mix 2 (shared by proj pairs, qkv chunks,
    warmup) = 8 banks.  No scoped-pool closes mid-kernel (they barrier).
  - proj psum->sbuf copies alternate DVE/ScalarE; y and rowsum stream out
    via DMA from SBUF.
"""

import numpy as np

N, C, D, H = 4096, 512, 64, 8
NB = 512              # query-block width
NBLK = N // NB        # 8 query blocks
MT = N // 128         # 32 key tiles
KO = C // 128         # 4 contraction tiles for the qkv projection
NG = MT // 2          # 16 key-tile groups per block (2 tiles / group)
LAG = 3               # av runs LAG group-slots behind scores/exp

SCHR_A = 128.0 / np.log(2.0)      # bf16-domain Schraudolph multiplier
SCHR_B = 127.0 * 128.0 - 5.5      # tuned offset (minimax ratio +-3.3%)

_CACHE = {}


def _build(scale: float):
    import concourse.mybir as mybir
    import concourse.tile as tile
    from concourse import bacc
    from concourse.bass import ts

    f32 = mybir.dt.float32
    bf16 = mybir.dt.bfloat16
    i16 = mybir.dt.int16
    Exp = mybir.ActivationFunctionType.Exp
    Copy = mybir.ActivationFunctionType.Copy
    Ident = mybir.ActivationFunctionType.Identity
    mult = mybir.AluOpType.mult
    add = mybir.AluOpType.add

    nc = bacc.Bacc("TRN2", target_bir_lowering=False, debug=False)

    xT = nc.dram_tensor("xT", [128, NBLK * KO * NB], bf16, kind="ExternalInput")
    wq = nc.dram_tensor("wqkvT", [128, KO * 192], bf16, kind="ExternalInput")
    bqk = nc.dram_tensor("bqkv", [2, 128], f32, kind="ExternalInput")
    wp = nc.dram_tensor("wprojT", [D, C], bf16, kind="ExternalInput")
    y = nc.dram_tensor("y", [N, C], f32, kind="ExternalOutput")
    rs = nc.dram_tensor("rowsum", [1, N], bf16, kind="ExternalOutput")

    with tile.TileContext(nc) as tc:
        with (
            tc.tile_pool(name="persist", bufs=1) as persist,
            tc.tile_pool(name="xpool", bufs=1) as xpool,
            tc.tile_pool(name="vstage", bufs=2) as vstage,
            tc.tile_pool(name="ps_sc", bufs=2, space="PSUM") as ps_sc,
            tc.tile_pool(name="ps_av", bufs=1, space="PSUM") as ps_av,
            tc.tile_pool(name="ps_mix", bufs=2, space="PSUM") as ps_mix,
            tc.tile_pool(name="sb_pt", bufs=4) as sb_pt,
            tc.tile_pool(name="sb_o", bufs=2) as sb_o,
            tc.tile_pool(name="sb_y", bufs=4) as sb_y,
        ):
            A = persist.tile([128, N], bf16)            # q^T: rows 0:64 + dup
            B = persist.tile([128, N], bf16)            # k^T: rows 64:128 + dup
            vTs = persist.tile([64, N], bf16)           # v^T staging
            v_sb = persist.tile([128, MT, 65], bf16)    # [v | 1] key tiles
            wq_sb = persist.tile([128, KO, 192], bf16)
            b_sb = persist.tile([128, 2], f32)
            wp_sb = persist.tile([128, C], bf16)

            nc.gpsimd.memset(v_sb[:, :, 64], 1.0)
            xT_sb = xpool.tile([128, NBLK, KO, NB], bf16)
            xT_r = xT.rearrange("p (b ko n) -> p b ko n", b=NBLK, ko=KO)
            nc.sync.dma_start(wq_sb[:], wq.rearrange("p (ko m) -> p ko m", ko=KO))
            nc.sync.dma_start(xT_sb[:, 0], xT_r[:, 0])
            nc.sync.dma_start(b_sb[:], bqk.rearrange("t p -> p t"))
            nc.sync.dma_start(xT_sb[:, 1], xT_r[:, 1])
            nc.sync.dma_start(xT_sb[:, 2], xT_r[:, 2])

            # HAM warmup: junk matmuls keep the PE busy through the initial
            # DMA window so the clock gate reaches 8/8 before real work.
            warm_src = persist.tile([128, NB], bf16)
            nc.vector.memset(warm_src[:], 0.5)
            wps = ps_mix.tile([128, NB], f32, tag="mix", name="warm")
            for _ in range(5):
                nc.tensor.matmul(
                    wps[:], warm_src[:, 0:128], warm_src[:], start=True, stop=True
                )

            def qk_chunk(nch):
                # [q|k] chunk: psum rows 0:64 = q^T, 64:128 = k^T.  Two
                # lane-aligned ScalarE moves (+bias), then DMA duplicates
                # each into the other partition half for row-packing.
                ps = ps_mix.tile([128, NB], f32, tag="mix", name="qk")
                for ko in range(KO):
                    nc.tensor.matmul(
                        ps[:],
                        wq_sb[:, ko, 0:128],
                        xT_sb[:, nch, ko, :],
                        start=(ko == 0),
                        stop=(ko == KO - 1),
                    )
                nc.scalar.activation(
                    A[0:64, ts(nch, NB)], ps[0:64], Ident, bias=b_sb[0:64, 0:1]
                )
                nc.scalar.activation(
                    B[64:128, ts(nch, NB)], ps[64:128], Ident,
                    bias=b_sb[64:128, 0:1],
                )
                nc.sync.dma_start(A[64:128, ts(nch, NB)], A[0:64, ts(nch, NB)])
                nc.sync.dma_start(B[0:64, ts(nch, NB)], B[64:128, ts(nch, NB)])

            def v_chunk(nch):
                ps = ps_mix.tile([128, NB], f32, tag="mix", name="v")
                for ko in range(KO):
                    nc.tensor.matmul(
                        ps[0:64],
                        wq_sb[:, ko, 128:192],
                        xT_sb[:, nch, ko, :],
                        start=(ko == 0),
                        stop=(ko == KO - 1),
                    )
                nc.scalar.activation(
                    vTs[0:64, ts(nch, NB)], ps[0:64], Ident,
                    bias=b_sb[0:64, 1:2],
                )
                # XBAR transpose into contiguous staging, then GpSimd
                # scatters into the stride-65 [v|1] tiles.
                vst = vstage.tile([128, 4, 64], bf16, tag="vst", name="vst")
                nc.sync.dma_start_transpose(vst[:], vTs[0:64, ts(nch, NB)])
                nc.gpsimd.tensor_copy(v_sb[:, 4 * nch : 4 * nch + 4, 0:64], vst[:])

            # pre-loop: qkv for blocks 0 and 1; blocks 2+ are JIT'd in-loop
            qk_chunk(0)
            v_chunk(0)
            nc.sync.dma_start(xT_sb[:, 3], xT_r[:, 3])
            qk_chunk(1)
            v_chunk(1)
            nc.sync.dma_start(wp_sb[0:64], wp[:])
            nc.sync.dma_start(wp_sb[64:128], wp[:])

            # all blocks run as interleaved PAIRS (2k, 2k+1); av matmuls
            # cover both blocks of a pair at N=1024, halving av LDW overhead
            seq = [
                (2 * k + p, g)
                for k in range(NBLK // 2)
                for g in range(NG)
                for p in (0, 1)
            ]
            avs = {}
            pts = {}
            outTs = {}
            pending_proj = []

            def exp_engine(i):
                return "A" if i % 2 == 0 else "D"

            def emit_scores(nb, g):
                sc = ps_sc.tile([128, 2 * NB], f32, tag="sc", name="sc")
                for j, mt in enumerate((2 * g, 2 * g + 1)):
                    half = 64 * (mt % 2)
                    nc.tensor.matmul(
                        sc[:, ts(j, NB)],
                        B[half : half + 64, ts(mt, 128)],
                        A[half : half + 64, ts(nb, NB)],
                        start=True,
                        stop=True,
                        tile_position=(half, 0),
                    )
                return sc

            def emit_exp(i, nb, g, sc):
                # pt layout per (pair, g): [128, tile j (2), block parity (2), NB]
                pair = nb // 2
                if (pair, g) not in pts:
                    pts[(pair, g)] = sb_pt.tile([128, 2, 2, NB], bf16, tag="pt", name="pt")
                pt = pts[(pair, g)]
                dst = pt[:, :, nb % 2, :]
                if exp_engine(i) == "A":
                    nc.scalar.activation(dst, sc[:], Exp, scale=scale)
                else:
                    nc.vector.tensor_scalar(
                        dst.bitcast(i16),
                        sc[:],
                        float(SCHR_A * scale),
                        float(SCHR_B),
                        mult,
                        add,
                    )

            def emit_av(pair, g):
                # one N=1024 matmul per key tile covers both blocks of a pair
                if g == 0:
                    avs[pair] = ps_av.tile([65, 2 * NB], f32, tag="av", name="av")
                pt = pts.pop((pair, g))
                for j, mt in enumerate((2 * g, 2 * g + 1)):
                    for p in (0, 1):
                        nc.tensor.matmul(
                            avs[pair][:, p * NB : (p + 1) * NB],
                            v_sb[:, mt, :],
                            pt[:, j, p, :],
                            start=(g == 0 and j == 0),
                            stop=(g == NG - 1 and j == 1),
                        )

            def emit_out(i, pair):
                # one copy moves out^T AND the bf16 rowsum row for both blocks
                outT = sb_o.tile([128, 2 * NB], bf16, tag="outT", name="outT")
                av = avs.pop(pair)
                nc.vector.tensor_copy(outT[0:65], av[:])
                nc.gpsimd.dma_start(rs[:, ts(pair, 2 * NB)], outT[64:65, :])
                # duplicate d-rows into the upper half (row-packed proj);
                # partition-shifting copies need DMA, after the rowsum flush
                nc.gpsimd.dma_start(outT[64:128], outT[0:64])
                outTs[pair] = outT
                for slot, (p, t) in enumerate(
                    ((0, 0), (1, 0), (0, 2), (1, 2))
                ):
                    pending_proj.append((i + 4 + 4 * slot, pair, p, t))

            def emit_proj_pair(pair, p, t):
                outT = outTs[pair]
                nb = 2 * pair + p
                off = p * NB
                ypA = ps_mix.tile([128, NB], f32, tag="mix", name="ypA")
                ypB = ps_mix.tile([128, NB], f32, tag="mix", name="ypB")
                nc.tensor.matmul(
                    ypA[:], outT[0:64, off + t * 128 : off + (t + 1) * 128],
                    wp_sb[0:64], start=True, stop=True,
                )
                nc.tensor.matmul(
                    ypB[:], outT[64:128, off + (t + 1) * 128 : off + (t + 2) * 128],
                    wp_sb[64:128], start=True, stop=True, tile_position=(64, 0),
                )
                for tt, ypx in ((t, ypA), (t + 1, ypB)):
                    ysb = sb_y.tile([128, NB], f32, tag="ysb", name="ysb")
                    if tt % 2 == 0:
                        nc.vector.tensor_copy(ysb[:], ypx[:])
                    else:
                        nc.scalar.activation(ysb[:], ypx[:], Copy)
                    row = nb * NB + tt * 128
                    nc.sync.dma_start(y[row : row + 128, :], ysb[:])
                if p == 1 and t == 2:
                    outTs.pop(pair)

            def flush_proj(i):
                while pending_proj and pending_proj[0][0] <= i:
                    _, pair, p, t = pending_proj.pop(0)
                    emit_proj_pair(pair, p, t)

            def emit_vwork(i):
                # front work during the first pair: chunks for blocks 2..7
                if i % 4 == 0 and 4 + i // 4 < NBLK:
                    nc.sync.dma_start(xT_sb[:, 4 + i // 4], xT_r[:, 4 + i // 4])
                if i % 4 == 0 and 2 + i // 4 < NBLK:
                    qk_chunk(2 + i // 4)
                if i % 4 == 2 and 2 + i // 4 < NBLK:
                    v_chunk(2 + i // 4)

            for i, (nb, g) in enumerate(seq):
                if i < 2 * NG:
                    emit_vwork(i)
                if i >= LAG and (i - LAG) % 2 == 1:
                    pnb, pg = seq[i - LAG]
                    emit_av(pnb // 2, pg)
                    if pg == NG - 1:
                        emit_out(i, pnb // 2)
                sc = emit_scores(nb, g)
                emit_exp(i, nb, g, sc)
                flush_proj(i)
            # tail: remaining avs, last pair's out + proj
            for i in range(len(seq) - LAG, len(seq)):
                pnb, pg = seq[i]
                if i % 2 == 1:
                    emit_av(pnb // 2, pg)
                    if pg == NG - 1:
                        emit_out(i + LAG, pnb // 2)
            flush_proj(10 ** 9)

    nc.compile()
    return nc


def _get_nc(scale: float):
    key = round(float(scale), 12)
    if key not in _CACHE:
        _CACHE[key] = _build(float(scale))
    return _CACHE[key]


def _prep_in_maps(x, Wqkv, bqkv, Wproj):
    import ml_dtypes

    bf = ml_dtypes.bfloat16
    x = np.asarray(x, np.float32).reshape(N, C)
    xT = np.ascontiguousarray(
        x.T.reshape(KO, 128, NBLK, NB).transpose(1, 2, 0, 3)
        .reshape(128, NBLK * KO * NB)
    ).astype(bf)
    Wqkv = np.asarray(Wqkv, np.float32)
    bqkv = np.asarray(bqkv, np.float32).reshape(3 * C)
    Wproj = np.asarray(Wproj, np.float32)
    in_maps = []
    for h in range(H):
        q = Wqkv[h * D : (h + 1) * D]
        k = Wqkv[C + h * D : C + (h + 1) * D]
        v = Wqkv[2 * C + h * D : 2 * C + (h + 1) * D]
        wqkvT = np.ascontiguousarray(
            np.concatenate([q, k, v], 0).T.reshape(KO, 128, 192)
            .transpose(1, 0, 2).reshape(128, KO * 192)
        ).astype(bf)
        bq = bqkv[h * D : (h + 1) * D]
        bk = bqkv[C + h * D : C + (h + 1) * D]
        bv = bqkv[2 * C + h * D : 2 * C + (h + 1) * D]
        bt = np.zeros((2, 128), np.float32)
        bt[0] = np.concatenate([bq, bk])
        bt[1, 0:64] = bv
        wprojT = np.ascontiguousarray(Wproj[:, h * D : (h + 1) * D].T).astype(bf)
        in_maps.append({"xT": xT, "wqkvT": wqkvT, "bqkv": bt, "wprojT": wprojT})
    return in_maps


def _finish(results, bproj):
    acc = np.zeros((N, C), np.float64)
    for h in range(H):
        yh = np.asarray(results[h]["y"], np.float64)
        rh = np.asarray(results[h]["rowsum"], np.float64).reshape(N)
        acc += yh / rh[:, None]
    acc += np.asarray(bproj, np.float64)
    return acc.reshape(1, 64, 64, C).astype(np.float32)


def _run(x, num_heads, bias, scale, Wqkv, bqkv, Wproj, bproj, trace=False):
    from concourse.bass_utils import run_bass_kernel_spmd

    assert int(num_heads) == H
    nc = _get_nc(float(scale))
    in_maps = _prep_in_maps(x, Wqkv, bqkv, Wproj)
    res = run_bass_kernel_spmd(nc, in_maps, core_ids=list(range(H)), trace=trace)
    return _finish(res.results, bproj), res


def kernel(x, num_heads, bias, scale, Wqkv, bqkv, Wproj, bproj):
    out, _ = _run(x, num_heads, bias, scale, Wqkv, bqkv, Wproj, bproj)
    return out


# revision 23
# speedup vs baseline: 1.0222x; 1.0222x over previous
"""Multi-head attention (B=1, N=4096, C=512, H=8) on 8 Trainium2 NeuronCores.

Tensor-parallel over heads: core h computes head h end-to-end (QKV proj,
softmax(q k^T) v, proj-slice), emitting the *unnormalized* projected partial
(softmax denominator deferred) plus per-query bf16 row sums; the host divides
and all-reduces (sums) the 8 partials and adds bproj.

Design:
  - whole PE path is bf16 (weights, x, q/k/v, exp output).  PSUM accums stay
    fp32.  End-to-end rel err ~1e-2 vs the 2e-2 gate.
  - softmax exp (16.7M elements/core, the largest single engine load) is
    split between ScalarE (exact Exp activation) and the Vector engine,
    which computes a Schraudolph-style exp in ONE op:
    pt_bf16 = bitcast(int16(s*scale*128/ln2 + (127*128-5.5)))
    (the int16 bit pattern of 2^y IS the bf16 value; linear-frac approx has
    max multiplicative error ~3.3%, harmless under softmax normalization).
  - q^T/k^T are computed once (single [q|k] M=128 chunk per block) and
    duplicated into both partition halves by SBUF-SBUF DMA, so score matmuls
    run 2-way row-packed (K=64 pairs).  No separate q pass.
  - v tiles [128 keys, 64] come from DMA XBAR transposes into a contiguous
    staging tile (the XBAR writer needs contiguous output), then GpSimd
    scatters into the stride-65 [v | 1] tiles (ones column -> free row sums,
    M=65 av matmuls).
  - blocks 0 and 1 run interleaved at group granularity so the k/v
    projection JIT work spreads over twice the steps; blocks 2-7 have their
    chunks prebuilt by then.
  - av runs LAG=3 flat steps behind scores/exp (two exp engines in flight),
    and is emitted BEFORE scores each step (the PE queue is in-order; av's
    inputs are always ready while scores may wait on the sc-psum recycle).
  - PSUM: sc 2x2 banks + av 2x1 + mix 2x1 (shared ring: qkv chunks, proj
    pairs, warmup) = 8 banks.  No scoped-pool closes mid-kernel (barriers).
  - proj psum->sbuf copies alternate DVE/ScalarE; y streams via sync DMA;
    rowsum + outT partition-dup go via GpSimd software DGE.
"""

import numpy as np

N, C, D, H = 4096, 512, 64, 8
NB = 512              # query-block width
NBLK = N // NB        # 8 query blocks
MT = N // 128         # 32 key tiles
KO = C // 128         # 4 contraction tiles for the qkv projection
NG = MT // 2          # 16 key-tile groups per block (2 tiles / group)
LAG = 3               # av runs LAG flat steps behind scores/exp

SCHR_A = 128.0 / np.log(2.0)      # bf16-domain Schraudolph multiplier
SCHR_B = 127.0 * 128.0 - 5.5      # tuned offset (minimax ratio +-3.3%)

_CACHE = {}


def _build(scale: float):
    import concourse.mybir as mybir
    import concourse.tile as tile
    from concourse import bacc
    from concourse.bass import ts

    f32 = mybir.dt.float32
    bf16 = mybir.dt.bfloat16
    i16 = mybir.dt.int16
    Exp = mybir.ActivationFunctionType.Exp
    Copy = mybir.ActivationFunctionType.Copy
    Ident = mybir.ActivationFunctionType.Identity
    mult = mybir.AluOpType.mult
    add = mybir.AluOpType.add

    nc = bacc.Bacc("TRN2", target_bir_lowering=False, debug=False)

    xT = nc.dram_tensor("xT", [128, KO * N], bf16, kind="ExternalInput")
    wq = nc.dram_tensor("wqkvT", [128, KO * 192], bf16, kind="ExternalInput")
    bqk = nc.dram_tensor("bqkv", [2, 128], f32, kind="ExternalInput")
    wp = nc.dram_tensor("wprojT", [D, C], bf16, kind="ExternalInput")
    y = nc.dram_tensor("y", [N, C], f32, kind="ExternalOutput")
    rs = nc.dram_tensor("rowsum", [1, N], bf16, kind="ExternalOutput")

    with tile.TileContext(nc) as tc:
        with (
            tc.tile_pool(name="persist", bufs=1) as persist,
            tc.tile_pool(name="xpool", bufs=1) as xpool,
            tc.tile_pool(name="vstage", bufs=2) as vstage,
            tc.tile_pool(name="ps_sc", bufs=2, space="PSUM") as ps_sc,
            tc.tile_pool(name="ps_av", bufs=2, space="PSUM") as ps_av,
            tc.tile_pool(name="ps_mix", bufs=2, space="PSUM") as ps_mix,
            tc.tile_pool(name="sb_pt", bufs=4) as sb_pt,
            tc.tile_pool(name="sb_o", bufs=2) as sb_o,
            tc.tile_pool(name="sb_y", bufs=4) as sb_y,
        ):
            A = persist.tile([128, N], bf16)            # q^T: rows 0:64 + dup
            B = persist.tile([128, N], bf16)            # k^T: rows 64:128 + dup
            vTs = persist.tile([64, N], bf16)           # v^T staging
            v_sb = persist.tile([128, MT, 65], bf16)    # [v | 1] key tiles
            wq_sb = persist.tile([128, KO, 192], bf16)
            b_sb = persist.tile([128, 2], f32)
            wp_sb = persist.tile([128, C], bf16)

            nc.gpsimd.memset(v_sb[:, :, 64], 1.0)
            xT_sb = xpool.tile([128, KO, N], bf16)
            xT_r = xT.rearrange("p (ko n) -> p ko n", ko=KO)
            nc.sync.dma_start(wq_sb[:], wq.rearrange("p (ko m) -> p ko m", ko=KO))
            nc.sync.dma_start(xT_sb[:, :, ts(0, NB)], xT_r[:, :, ts(0, NB)])
            nc.sync.dma_start(b_sb[:], bqk.rearrange("t p -> p t"))
            for nch in range(1, 4):
                nc.sync.dma_start(xT_sb[:, :, ts(nch, NB)], xT_r[:, :, ts(nch, NB)])
            nc.sync.dma_start(wp_sb[0:64], wp[:])
            nc.sync.dma_start(wp_sb[64:128], wp[:])
            for nch in range(4, NBLK):
                nc.sync.dma_start(xT_sb[:, :, ts(nch, NB)], xT_r[:, :, ts(nch, NB)])

            # HAM warmup: junk matmuls keep the PE busy through the initial
            # DMA window so the clock gate reaches 8/8 before real work.
            warm_src = persist.tile([128, NB], bf16)
            nc.vector.memset(warm_src[:], 0.5)
            wps = ps_mix.tile([128, NB], f32, tag="mix", name="warm")
            for _ in range(6):
                nc.tensor.matmul(
                    wps[:], warm_src[:, 0:128], warm_src[:], start=True, stop=True
                )

            def qk_chunk(nch):
                # [q|k] chunk: psum rows 0:64 = q^T, 64:128 = k^T.  Two
                # lane-aligned ScalarE moves (+bias), then DMA duplicates
                # each into the other partition half for row-packing.
                ps = ps_mix.tile([128, NB], f32, tag="mix", name="qk")
                for ko in range(KO):
                    nc.tensor.matmul(
                        ps[:],
                        wq_sb[:, ko, 0:128],
                        xT_sb[:, ko, ts(nch, NB)],
                        start=(ko == 0),
                        stop=(ko == KO - 1),
                    )
                nc.scalar.activation(
                    A[0:64, ts(nch, NB)], ps[0:64], Ident, bias=b_sb[0:64, 0:1]
                )
                nc.scalar.activation(
                    B[64:128, ts(nch, NB)], ps[64:128], Ident,
                    bias=b_sb[64:128, 0:1],
                )
                nc.sync.dma_start(A[64:128, ts(nch, NB)], A[0:64, ts(nch, NB)])
                nc.sync.dma_start(B[0:64, ts(nch, NB)], B[64:128, ts(nch, NB)])

            def v_chunk(nch):
                ps = ps_mix.tile([128, NB], f32, tag="mix", name="v")
                for ko in range(KO):
                    nc.tensor.matmul(
                        ps[0:64],
                        wq_sb[:, ko, 128:192],
                        xT_sb[:, ko, ts(nch, NB)],
                        start=(ko == 0),
                        stop=(ko == KO - 1),
                    )
                nc.scalar.activation(
                    vTs[0:64, ts(nch, NB)], ps[0:64], Ident,
                    bias=b_sb[0:64, 1:2],
                )
                # XBAR transpose into contiguous staging, then GpSimd
                # scatters into the stride-65 [v|1] tiles.
                vst = vstage.tile([128, 4, 64], bf16, tag="vst", name="vst")
                nc.sync.dma_start_transpose(vst[:], vTs[0:64, ts(nch, NB)])
                nc.gpsimd.tensor_copy(v_sb[:, 4 * nch : 4 * nch + 4, 0:64], vst[:])

            # pre-loop: qkv for tiles 0-3 only; the rest is JIT in-loop
            qk_chunk(0)
            v_chunk(0)

            seq = [(nb, g) for g in range(NG) for nb in (0, 1)]
            seq += [(nb, g) for nb in range(2, NBLK) for g in range(NG)]
            avs = {}
            pts = {}
            outTs = {}
            pending_proj = []

            def exp_engine(i):
                return "A" if i % 2 == 0 else "D"

            def emit_scores(nb, g):
                sc = ps_sc.tile([128, 2 * NB], f32, tag="sc", name="sc")
                for j, mt in enumerate((2 * g, 2 * g + 1)):
                    half = 64 * (mt % 2)
                    nc.tensor.matmul(
                        sc[:, ts(j, NB)],
                        B[half : half + 64, ts(mt, 128)],
                        A[half : half + 64, ts(nb, NB)],
                        start=True,
                        stop=True,
                        tile_position=(half, 0),
                    )
                return sc

            def emit_exp(i, nb, g, sc):
                pt = sb_pt.tile([128, 2 * NB], bf16, tag="pt", name="pt")
                if exp_engine(i) == "A":
                    nc.scalar.activation(pt[:], sc[:], Exp, scale=scale)
                else:
                    nc.vector.tensor_scalar(
                        pt[:].bitcast(i16),
                        sc[:],
                        float(SCHR_A * scale),
                        float(SCHR_B),
                        mult,
                        add,
                    )
                pts[(nb, g)] = pt

            def emit_av(nb, g):
                if g == 0:
                    avs[nb] = ps_av.tile([65, NB], f32, tag="av", name="av")
                pt = pts.pop((nb, g))
                for j, mt in enumerate((2 * g, 2 * g + 1)):
                    nc.tensor.matmul(
                        avs[nb][:],
                        v_sb[:, mt, :],
                        pt[:, ts(j, NB)],
                        start=(g == 0 and j == 0),
                        stop=(g == NG - 1 and j == 1),
                    )

            def emit_out(i, nb):
                # one DVE copy moves out^T AND the bf16 rowsum row
                outT = sb_o.tile([128, NB], bf16, tag="outT", name="outT")
                av = avs.pop(nb)
                nc.vector.tensor_copy(outT[0:65], av[:])
                nc.gpsimd.dma_start(rs[:, ts(nb, NB)], outT[64:65, :])
                # duplicate d-rows into the upper half (row-packed proj);
                # partition-shifting copies need DMA, after the rowsum flush
                nc.gpsimd.dma_start(outT[64:128], outT[0:64])
                outTs[nb] = outT
                pending_proj.append((i + 4, nb, 0))
                pending_proj.append((i + 7, nb, 2))

            def emit_proj_pair(nb, t):
                outT = outTs[nb]
                ypA = ps_mix.tile([128, NB], f32, tag="mix", name="ypA")
                ypB = ps_mix.tile([128, NB], f32, tag="mix", name="ypB")
                nc.tensor.matmul(
                    ypA[:], outT[0:64, ts(t, 128)], wp_sb[0:64],
                    start=True, stop=True,
                )
                nc.tensor.matmul(
                    ypB[:], outT[64:128, ts(t + 1, 128)], wp_sb[64:128],
                    start=True, stop=True, tile_position=(64, 0),
                )
                for tt, ypx in ((t, ypA), (t + 1, ypB)):
                    ysb = sb_y.tile([128, NB], f32, tag="ysb", name="ysb")
                    if tt % 2 == 0:
                        nc.vector.tensor_copy(ysb[:], ypx[:])
                    else:
                        nc.scalar.activation(ysb[:], ypx[:], Copy)
                    row = nb * NB + tt * 128
                    nc.sync.dma_start(y[row : row + 128, :], ysb[:])
                if t == 2:
                    outTs.pop(nb)

            def flush_proj(i):
                while pending_proj and pending_proj[0][0] <= i:
                    _, nb, t = pending_proj.pop(0)
                    emit_proj_pair(nb, t)

            def emit_vwork(i):
                # JIT front work during the interleaved 0/1 phase: chunks
                # for blocks 1..7, one qk/v pair every 4 flat steps
                if i % 4 == 0 and 1 + i // 4 < NBLK:
                    qk_chunk(1 + i // 4)
                if i % 4 == 2 and 1 + i // 4 < NBLK:
                    v_chunk(1 + i // 4)

            for i, (nb, g) in enumerate(seq):
                if i < 2 * NG:
                    emit_vwork(i)
                if i >= LAG:
                    pnb, pg = seq[i - LAG]
                    emit_av(pnb, pg)
                    if pg == NG - 1:
                        emit_out(i, pnb)
                sc = emit_scores(nb, g)
                emit_exp(i, nb, g, sc)
                flush_proj(i)
            # tail: last LAG groups' av, last block's out + proj
            for i in range(len(seq) - LAG, len(seq)):
                pnb, pg = seq[i]
                emit_av(pnb, pg)
                if pg == NG - 1:
                    emit_out(i + LAG, pnb)
            flush_proj(10 ** 9)

    nc.compile()
    return nc


def _get_nc(scale: float):
    key = round(float(scale), 12)
    if key not in _CACHE:
        _CACHE[key] = _build(float(scale))
    return _CACHE[key]


def _prep_in_maps(x, Wqkv, bqkv, Wproj):
    import ml_dtypes

    bf = ml_dtypes.bfloat16
    x = np.asarray(x, np.float32).reshape(N, C)
    xT = np.ascontiguousarray(
        x.T.reshape(KO, 128, N).transpose(1, 0, 2).reshape(128, KO * N)
    ).astype(bf)
    Wqkv = np.asarray(Wqkv, np.float32)
    bqkv = np.asarray(bqkv, np.float32).reshape(3 * C)
    Wproj = np.asarray(Wproj, np.float32)
    in_maps = []
    for h in range(H):
        q = Wqkv[h * D : (h + 1) * D]
        k = Wqkv[C + h * D : C + (h + 1) * D]
        v = Wqkv[2 * C + h * D : 2 * C + (h + 1) * D]
        wqkvT = np.ascontiguousarray(
            np.concatenate([q, k, v], 0).T.reshape(KO, 128, 192)
            .transpose(1, 0, 2).reshape(128, KO * 192)
        ).astype(bf)
        bq = bqkv[h * D : (h + 1) * D]
        bk = bqkv[C + h * D : C + (h + 1) * D]
        bv = bqkv[2 * C + h * D : 2 * C + (h + 1) * D]
        bt = np.zeros((2, 128), np.float32)
        bt[0] = np.concatenate([bq, bk])
        bt[1, 0:64] = bv
        wprojT = np.ascontiguousarray(Wproj[:, h * D : (h + 1) * D].T).astype(bf)
        in_maps.append({"xT": xT, "wqkvT": wqkvT, "bqkv": bt, "wprojT": wprojT})
    return in_maps


def _finish(results, bproj):
    acc = np.zeros((N, C), np.float64)
    for h in range(H):
        yh = np.asarray(results[h]["y"], np.float64)
        rh = np.asarray(results[h]["rowsum"], np.float64).reshape(N)
        acc += yh / rh[:, None]
    acc += np.asarray(bproj, np.float64)
    return acc.reshape(1, 64, 64, C).astype(np.float32)


def _run(x, num_heads, bias, scale, Wqkv, bqkv, Wproj, bproj, trace=False):
    from concourse.bass_utils import run_bass_kernel_spmd

    assert int(num_heads) == H
    nc = _get_nc(float(scale))
    in_maps = _prep_in_maps(x, Wqkv, bqkv, Wproj)
    res = run_bass_kernel_spmd(nc, in_maps, core_ids=list(range(H)), trace=trace)
    return _finish(res.results, bproj), res


def kernel(x, num_heads, bias, scale, Wqkv, bqkv, Wproj, bproj):
    out, _ = _run(x, num_heads, bias, scale, Wqkv, bqkv, Wproj, bproj)
    return out


# revision 24
# speedup vs baseline: 1.0349x; 1.0124x over previous
"""Multi-head attention (B=1, N=4096, C=512, H=8) on 8 Trainium2 NeuronCores.

Tensor-parallel over heads: core h computes head h end-to-end (QKV proj,
softmax(q k^T) v, proj-slice), emitting the *unnormalized* projected partial
(softmax denominator deferred) plus per-query bf16 row sums; the host divides
and all-reduces (sums) the 8 partials and adds bproj.

Design:
  - whole PE path is bf16 (weights, x, q/k/v, exp output).  PSUM accums stay
    fp32.  End-to-end rel err ~1e-2 vs the 2e-2 gate.
  - softmax exp (16.7M elements/core, the largest single engine load) is
    split between ScalarE (exact Exp activation) and the Vector engine,
    which computes a Schraudolph-style exp in ONE op:
    pt_bf16 = bitcast(int16(s*scale*128/ln2 + (127*128-5.5)))
    (the int16 bit pattern of 2^y IS the bf16 value; linear-frac approx has
    max multiplicative error ~3.3%, harmless under softmax normalization).
  - q^T/k^T are computed once (single [q|k] M=128 chunk per block) and
    duplicated into both partition halves by SBUF-SBUF DMA, so score matmuls
    run 2-way row-packed (K=64 pairs).  No separate q pass.
  - v tiles [128 keys, 64] come from DMA XBAR transposes into a contiguous
    staging tile (the XBAR writer needs contiguous output), then GpSimd
    scatters into the stride-65 [v | 1] tiles (ones column -> free row sums,
    M=65 av matmuls).
  - blocks 0 and 1 run interleaved at group granularity so the k/v
    projection JIT work spreads over twice the steps; blocks 2-7 have their
    chunks prebuilt by then.
  - av runs LAG=3 flat steps behind scores/exp (two exp engines in flight),
    and is emitted BEFORE scores each step (the PE queue is in-order; av's
    inputs are always ready while scores may wait on the sc-psum recycle).
  - PSUM: sc 2x2 banks + av 2x1 + mix 2x1 (shared ring: qkv chunks, proj
    pairs, warmup) = 8 banks.  No scoped-pool closes mid-kernel (barriers).
  - proj psum->sbuf copies alternate DVE/ScalarE; y streams via sync DMA;
    rowsum + outT partition-dup go via GpSimd software DGE.
"""

import numpy as np

N, C, D, H = 4096, 512, 64, 8
NB = 512              # query-block width
NBLK = N // NB        # 8 query blocks
MT = N // 128         # 32 key tiles
KO = C // 128         # 4 contraction tiles for the qkv projection
NG = MT // 2          # 16 key-tile groups per block (2 tiles / group)
LAG = 4               # av runs LAG flat steps behind scores/exp

SCHR_A = 128.0 / np.log(2.0)      # bf16-domain Schraudolph multiplier
SCHR_B = 127.0 * 128.0 - 5.5      # tuned offset (minimax ratio +-3.3%)

_CACHE = {}


def _build(scale: float):
    import concourse.mybir as mybir
    import concourse.tile as tile
    from concourse import bacc
    from concourse.bass import ts

    f32 = mybir.dt.float32
    bf16 = mybir.dt.bfloat16
    i16 = mybir.dt.int16
    Exp = mybir.ActivationFunctionType.Exp
    Copy = mybir.ActivationFunctionType.Copy
    Ident = mybir.ActivationFunctionType.Identity
    mult = mybir.AluOpType.mult
    add = mybir.AluOpType.add

    nc = bacc.Bacc("TRN2", target_bir_lowering=False, debug=False)

    xT = nc.dram_tensor("xT", [128, KO * N], bf16, kind="ExternalInput")
    wq = nc.dram_tensor("wqkvT", [128, KO * 192], bf16, kind="ExternalInput")
    bqk = nc.dram_tensor("bqkv", [2, 128], f32, kind="ExternalInput")
    wp = nc.dram_tensor("wprojT", [D, C], bf16, kind="ExternalInput")
    y = nc.dram_tensor("y", [N, C], f32, kind="ExternalOutput")
    rs = nc.dram_tensor("rowsum", [1, N], bf16, kind="ExternalOutput")

    with tile.TileContext(nc) as tc:
        with (
            tc.tile_pool(name="persist", bufs=1) as persist,
            tc.tile_pool(name="xpool", bufs=1) as xpool,
            tc.tile_pool(name="vstage", bufs=2) as vstage,
            tc.tile_pool(name="ps_sc", bufs=2, space="PSUM") as ps_sc,
            tc.tile_pool(name="ps_av", bufs=2, space="PSUM") as ps_av,
            tc.tile_pool(name="ps_mix", bufs=2, space="PSUM") as ps_mix,
            tc.tile_pool(name="sb_pt", bufs=5) as sb_pt,
            tc.tile_pool(name="sb_o", bufs=3) as sb_o,
            tc.tile_pool(name="sb_y", bufs=6) as sb_y,
        ):
            A = persist.tile([128, N], bf16)            # q^T: rows 0:64 + dup
            B = persist.tile([128, N], bf16)            # k^T: rows 64:128 + dup
            vTs = persist.tile([64, N], bf16)           # v^T staging
            v_sb = persist.tile([128, MT, 65], bf16)    # [v | 1] key tiles
            wq_sb = persist.tile([128, KO, 192], bf16)
            b_sb = persist.tile([128, 2], f32)
            wp_sb = persist.tile([128, C], bf16)

            nc.gpsimd.memset(v_sb[:, :, 64], 1.0)
            xT_sb = xpool.tile([128, KO, N], bf16)
            xT_r = xT.rearrange("p (ko n) -> p ko n", ko=KO)
            nc.sync.dma_start(wq_sb[:], wq.rearrange("p (ko m) -> p ko m", ko=KO))
            nc.sync.dma_start(xT_sb[:, :, ts(0, NB)], xT_r[:, :, ts(0, NB)])
            nc.sync.dma_start(b_sb[:], bqk.rearrange("t p -> p t"))
            for nch in range(1, 4):
                nc.sync.dma_start(xT_sb[:, :, ts(nch, NB)], xT_r[:, :, ts(nch, NB)])
            nc.sync.dma_start(wp_sb[0:64], wp[:])
            nc.sync.dma_start(wp_sb[64:128], wp[:])
            for nch in range(4, NBLK):
                nc.sync.dma_start(xT_sb[:, :, ts(nch, NB)], xT_r[:, :, ts(nch, NB)])

            # HAM warmup: junk matmuls keep the PE busy through the initial
            # DMA window so the clock gate reaches 8/8 before real work.
            warm_src = persist.tile([128, NB], bf16)
            nc.vector.memset(warm_src[:], 0.5)
            wps = ps_mix.tile([128, NB], f32, tag="mix", name="warm")
            for _ in range(6):
                nc.tensor.matmul(
                    wps[:], warm_src[:, 0:128], warm_src[:], start=True, stop=True
                )

            def qk_chunk(nch):
                # [q|k] chunk: psum rows 0:64 = q^T, 64:128 = k^T.  Two
                # lane-aligned ScalarE moves (+bias), then DMA duplicates
                # each into the other partition half for row-packing.
                ps = ps_mix.tile([128, NB], f32, tag="mix", name="qk")
                for ko in range(KO):
                    nc.tensor.matmul(
                        ps[:],
                        wq_sb[:, ko, 0:128],
                        xT_sb[:, ko, ts(nch, NB)],
                        start=(ko == 0),
                        stop=(ko == KO - 1),
                    )
                nc.scalar.activation(
                    A[0:64, ts(nch, NB)], ps[0:64], Ident, bias=b_sb[0:64, 0:1]
                )
                nc.scalar.activation(
                    B[64:128, ts(nch, NB)], ps[64:128], Ident,
                    bias=b_sb[64:128, 0:1],
                )
                nc.sync.dma_start(A[64:128, ts(nch, NB)], A[0:64, ts(nch, NB)])
                nc.sync.dma_start(B[0:64, ts(nch, NB)], B[64:128, ts(nch, NB)])

            def v_chunk(nch):
                ps = ps_mix.tile([128, NB], f32, tag="mix", name="v")
                for ko in range(KO):
                    nc.tensor.matmul(
                        ps[0:64],
                        wq_sb[:, ko, 128:192],
                        xT_sb[:, ko, ts(nch, NB)],
                        start=(ko == 0),
                        stop=(ko == KO - 1),
                    )
                nc.scalar.activation(
                    vTs[0:64, ts(nch, NB)], ps[0:64], Ident,
                    bias=b_sb[0:64, 1:2],
                )
                # XBAR transpose into contiguous staging, then GpSimd
                # scatters into the stride-65 [v|1] tiles.
                vst = vstage.tile([128, 4, 64], bf16, tag="vst", name="vst")
                nc.sync.dma_start_transpose(vst[:], vTs[0:64, ts(nch, NB)])
                nc.gpsimd.tensor_copy(v_sb[:, 4 * nch : 4 * nch + 4, 0:64], vst[:])

            # pre-loop: qkv for tiles 0-3 only; the rest is JIT in-loop
            qk_chunk(0)
            v_chunk(0)

            seq = [(nb, g) for g in range(NG) for nb in (0, 1)]
            seq += [(nb, g) for nb in range(2, NBLK) for g in range(NG)]
            avs = {}
            pts = {}
            outTs = {}
            pending_proj = []

            def exp_engine(i):
                return "A" if i % 2 == 0 else "D"

            def emit_scores(nb, g):
                sc = ps_sc.tile([128, 2 * NB], f32, tag="sc", name="sc")
                for j, mt in enumerate((2 * g, 2 * g + 1)):
                    half = 64 * (mt % 2)
                    nc.tensor.matmul(
                        sc[:, ts(j, NB)],
                        B[half : half + 64, ts(mt, 128)],
                        A[half : half + 64, ts(nb, NB)],
                        start=True,
                        stop=True,
                        tile_position=(half, 0),
                    )
                return sc

            def emit_exp(i, nb, g, sc):
                pt = sb_pt.tile([128, 2 * NB], bf16, tag="pt", name="pt")
                if exp_engine(i) == "A":
                    nc.scalar.activation(pt[:], sc[:], Exp, scale=scale)
                else:
                    nc.vector.tensor_scalar(
                        pt[:].bitcast(i16),
                        sc[:],
                        float(SCHR_A * scale),
                        float(SCHR_B),
                        mult,
                        add,
                    )
                pts[(nb, g)] = pt

            def emit_av(nb, g):
                if g == 0:
                    avs[nb] = ps_av.tile([65, NB], f32, tag="av", name="av")
                pt = pts.pop((nb, g))
                for j, mt in enumerate((2 * g, 2 * g + 1)):
                    nc.tensor.matmul(
                        avs[nb][:],
                        v_sb[:, mt, :],
                        pt[:, ts(j, NB)],
                        start=(g == 0 and j == 0),
                        stop=(g == NG - 1 and j == 1),
                    )

            def emit_out(i, nb):
                # one DVE copy moves out^T AND the bf16 rowsum row
                outT = sb_o.tile([128, NB], bf16, tag="outT", name="outT")
                av = avs.pop(nb)
                nc.vector.tensor_copy(outT[0:65], av[:])
                nc.gpsimd.dma_start(rs[:, ts(nb, NB)], outT[64:65, :])
                # duplicate d-rows into the upper half (row-packed proj);
                # partition-shifting copies need DMA, after the rowsum flush
                nc.gpsimd.dma_start(outT[64:128], outT[0:64])
                outTs[nb] = outT
                pending_proj.append((i + 4, nb, 0))
                pending_proj.append((i + 7, nb, 2))

            def emit_proj_pair(nb, t):
                outT = outTs[nb]
                ypA = ps_mix.tile([128, NB], f32, tag="mix", name="ypA")
                ypB = ps_mix.tile([128, NB], f32, tag="mix", name="ypB")
                nc.tensor.matmul(
                    ypA[:], outT[0:64, ts(t, 128)], wp_sb[0:64],
                    start=True, stop=True,
                )
                nc.tensor.matmul(
                    ypB[:], outT[64:128, ts(t + 1, 128)], wp_sb[64:128],
                    start=True, stop=True, tile_position=(64, 0),
                )
                for tt, ypx in ((t, ypA), (t + 1, ypB)):
                    ysb = sb_y.tile([128, NB], f32, tag="ysb", name="ysb")
                    if tt % 2 == 0:
                        nc.vector.tensor_copy(ysb[:], ypx[:])
                    else:
                        nc.scalar.activation(ysb[:], ypx[:], Copy)
                    row = nb * NB + tt * 128
                    nc.sync.dma_start(y[row : row + 128, :], ysb[:])
                if t == 2:
                    outTs.pop(nb)

            def flush_proj(i):
                while pending_proj and pending_proj[0][0] <= i:
                    _, nb, t = pending_proj.pop(0)
                    emit_proj_pair(nb, t)

            def emit_vwork(i):
                # JIT front work during the interleaved 0/1 phase: chunks
                # for blocks 1..7, one qk/v pair every 4 flat steps
                if i % 4 == 0 and 1 + i // 4 < NBLK:
                    qk_chunk(1 + i // 4)
                if i % 4 == 2 and 1 + i // 4 < NBLK:
                    v_chunk(1 + i // 4)

            for i, (nb, g) in enumerate(seq):
                if i < 2 * NG:
                    emit_vwork(i)
                if i >= LAG:
                    pnb, pg = seq[i - LAG]
                    emit_av(pnb, pg)
                    if pg == NG - 1:
                        emit_out(i, pnb)
                sc = emit_scores(nb, g)
                emit_exp(i, nb, g, sc)
                flush_proj(i)
            # tail: last LAG groups' av, last block's out + proj
            for i in range(len(seq) - LAG, len(seq)):
                pnb, pg = seq[i]
                emit_av(pnb, pg)
                if pg == NG - 1:
                    emit_out(i + LAG, pnb)
            flush_proj(10 ** 9)

    nc.compile()
    return nc


def _get_nc(scale: float):
    key = round(float(scale), 12)
    if key not in _CACHE:
        _CACHE[key] = _build(float(scale))
    return _CACHE[key]


def _prep_in_maps(x, Wqkv, bqkv, Wproj):
    import ml_dtypes

    bf = ml_dtypes.bfloat16
    x = np.asarray(x, np.float32).reshape(N, C)
    xT = np.ascontiguousarray(
        x.T.reshape(KO, 128, N).transpose(1, 0, 2).reshape(128, KO * N)
    ).astype(bf)
    Wqkv = np.asarray(Wqkv, np.float32)
    bqkv = np.asarray(bqkv, np.float32).reshape(3 * C)
    Wproj = np.asarray(Wproj, np.float32)
    in_maps = []
    for h in range(H):
        q = Wqkv[h * D : (h + 1) * D]
        k = Wqkv[C + h * D : C + (h + 1) * D]
        v = Wqkv[2 * C + h * D : 2 * C + (h + 1) * D]
        wqkvT = np.ascontiguousarray(
            np.concatenate([q, k, v], 0).T.reshape(KO, 128, 192)
            .transpose(1, 0, 2).reshape(128, KO * 192)
        ).astype(bf)
        bq = bqkv[h * D : (h + 1) * D]
        bk = bqkv[C + h * D : C + (h + 1) * D]
        bv = bqkv[2 * C + h * D : 2 * C + (h + 1) * D]
        bt = np.zeros((2, 128), np.float32)
        bt[0] = np.concatenate([bq, bk])
        bt[1, 0:64] = bv
        wprojT = np.ascontiguousarray(Wproj[:, h * D : (h + 1) * D].T).astype(bf)
        in_maps.append({"xT": xT, "wqkvT": wqkvT, "bqkv": bt, "wprojT": wprojT})
    return in_maps


def _finish(results, bproj):
    acc = np.zeros((N, C), np.float64)
    for h in range(H):
        yh = np.asarray(results[h]["y"], np.float64)
        rh = np.asarray(results[h]["rowsum"], np.float64).reshape(N)
        acc += yh / rh[:, None]
    acc += np.asarray(bproj, np.float64)
    return acc.reshape(1, 64, 64, C).astype(np.float32)


def _run(x, num_heads, bias, scale, Wqkv, bqkv, Wproj, bproj, trace=False):
    from concourse.bass_utils import run_bass_kernel_spmd

    assert int(num_heads) == H
    nc = _get_nc(float(scale))
    in_maps = _prep_in_maps(x, Wqkv, bqkv, Wproj)
    res = run_bass_kernel_spmd(nc, in_maps, core_ids=list(range(H)), trace=trace)
    return _finish(res.results, bproj), res


def kernel(x, num_heads, bias, scale, Wqkv, bqkv, Wproj, bproj):
    out, _ = _run(x, num_heads, bias, scale, Wqkv, bqkv, Wproj, bproj)
    return out
